# revision 2
# baseline (speedup 1.0000x reference)
"""Trainium2 Bass kernel for nn_KERN_21680994910746 (nms_detection).

Returns (obj_dists2, obj_preds, rel_dists), matching the reference.

Device work (8 NeuronCores, SPMD):
  - rel_dists = vr @ W_vr.T + b_vr   [16384, 51]
      sharded over vr rows (2048/core). vr is pre-transposed on host and
      split into two fp16 pieces (hi/lo) so the PE runs at full rate with
      ~1e-6 relative error: vr@W ~= vh@Wh + vl@Wh + vh@Wl.
  - per-class NMS pairwise IoU>0.3 masks  [classes x 512 x 512]
      sharded over classes (19/core, classes 1..150 + padding).
      Pairwise matrices are built on the TensorEngine as rank-2 (K=6, bf16
      3-piece exact coordinate splits) matmuls:
        A = x2_i - x2_j, B = x1_j - x1_i, C = y2_i - y2_j, D = y1_j - y1_i,
        E = 0.3*area_i + 0.3*area_j
      then w13 = relu(1.3*w_i - 1.3*(relu(A)+relu(B))),
           h   = relu(h_i - (relu(C)+relu(D))),
           mask = w13*h > E          (equivalent to IoU > 0.3)
      Only the upper block-triangle is computed (the mask is symmetric).

Host work (cheap or serial-scalar, as in production NMS implementations):
  input layout prep, softmax+argsort ordering (bit-identical to the
  reference via jax-on-CPU), the greedy NMS resolve over device-computed
  masks, and the final masked argmax.
"""

import os
import sys

import numpy as np

for _p in ("/opt/trn_rl_repo", "/root/.axon_site/_ro/trn_rl_repo", "/root/.axon_site"):
    if os.path.isdir(_p) and _p not in sys.path:
        sys.path.append(_p)

N_CORES = 8
N_OBJ, N_REL, C_OBJ, C_REL, D = 512, 16384, 151, 51, 4096
NMS_THRESH = np.float32(0.3)
CLS_PER_CORE = 19  # 8*19 = 152 slots for classes 1..150 (+2 padding)
ROWS_PER_CORE = N_REL // N_CORES  # 2048
_BLK = 128
_NBLK = N_OBJ // _BLK  # 4

_cache = {}


def _split3_bf16(x32):
    """Exact 3-piece bf16 split: x == p0 + p1 + p2 in f32."""
    import ml_dtypes

    x = np.asarray(x32, np.float32)
    p0 = x.astype(ml_dtypes.bfloat16)
    r1 = x - p0.astype(np.float32)
    p1 = r1.astype(ml_dtypes.bfloat16)
    r2 = r1 - p1.astype(np.float32)
    p2 = r2.astype(ml_dtypes.bfloat16)
    return (p0.astype(np.float32), p1.astype(np.float32), p2.astype(np.float32))


def _build_nc():
    import concourse.bacc as bacc
    import concourse.tile as tile
    import concourse.mybir as mybir

    f32 = mybir.dt.float32
    f16 = mybir.dt.float16
    bf16 = mybir.dt.bfloat16
    u8 = mybir.dt.uint8
    Alu = mybir.AluOpType
    Act = mybir.ActivationFunctionType

    nc = bacc.Bacc("TRN2", target_bir_lowering=False, debug=False,
                   num_devices=N_CORES)

    # ---- I/O ----
    vst = nc.dram_tensor("vst", [2 * D, ROWS_PER_CORE], f16,
                         kind="ExternalInput").ap()  # [vh; vl] of vrT slice
    wh = nc.dram_tensor("wh", [_BLK, D // _BLK, C_REL], f16,
                        kind="ExternalInput").ap()
    wl = nc.dram_tensor("wl", [_BLK, D // _BLK, C_REL], f16,
                        kind="ExternalInput").ap()
    bvr = nc.dram_tensor("bvr", [C_REL, 1], f32, kind="ExternalInput").ap()
    # rank-2 operand banks per class: [6(K), 10, 512] bf16
    pair = nc.dram_tensor("pair", [CLS_PER_CORE, 6, 10, N_OBJ], bf16,
                          kind="ExternalInput").ap()
    # ACT biases: [128, 4(bi), 19(cls), 2] : 1.3*wi | hi
    bias = nc.dram_tensor("bias", [_BLK, _NBLK, CLS_PER_CORE, 2], f32,
                          kind="ExternalInput").ap()

    relt = nc.dram_tensor("relt", [C_REL, ROWS_PER_CORE], f32,
                          kind="ExternalOutput").ap()
    masks = nc.dram_tensor("masks", [CLS_PER_CORE, N_OBJ, N_OBJ], u8,
                           kind="ExternalOutput").ap()

    vst_t = vst.rearrange("(ko p) m -> ko p m", p=_BLK)  # [64, 128, 2048]
    n_ko = 2 * D // _BLK  # 64 k-chunks in the [vh; vl] stack
    n_rb = ROWS_PER_CORE // 512  # 4 row-blocks

    with tile.TileContext(nc) as tc:
        with (
            tc.tile_pool(name="const", bufs=1) as constp,
            tc.tile_pool(name="pairp", bufs=2) as pairp,
            tc.tile_pool(name="vrp", bufs=1) as vrp,
            tc.tile_pool(name="work", bufs=3) as work,
            tc.tile_pool(name="outp", bufs=2) as outp,
            tc.tile_pool(name="mps", bufs=2, space="PSUM") as mps,
            tc.tile_pool(name="pps", bufs=6, space="PSUM") as pps,
        ):
            whs = constp.tile([_BLK, D // _BLK, C_REL], f16, tag="wh")
            wls = constp.tile([_BLK, D // _BLK, C_REL], f16, tag="wl")
            bvs = constp.tile([C_REL, 1], f32, tag="bv")
            bis = constp.tile([_BLK, _NBLK, CLS_PER_CORE, 2], f32, tag="bias")
            nc.sync.dma_start(whs[:], wh[:])
            nc.sync.dma_start(wls[:], wl[:])
            nc.sync.dma_start(bvs[:], bvr[:])
            nc.sync.dma_start(bis[:], bias[:])

            # ---------------- rel matmul ----------------
            for rb in range(n_rb):
                ps = mps.tile([C_REL, 512], f32, tag="relps")
                vres = []
                for ko in range(n_ko):
                    vt = vrp.tile([_BLK, 512], f16, tag=f"v{ko}")
                    nc.sync.dma_start(vt[:], vst_t[ko, :, rb * 512:(rb + 1) * 512])
                    vres.append(vt)
                    nc.tensor.matmul(ps[:], whs[:, ko % 32, :], vt[:],
                                     start=(ko == 0), stop=False)
                for ko in range(32):  # vh against Wl
                    nc.tensor.matmul(ps[:], wls[:, ko, :], vres[ko][:],
                                     start=False, stop=(ko == 31))
                ot = outp.tile([C_REL, 512], f32, tag="relout")
                nc.scalar.activation(ot[:], ps[:], Act.Identity,
                                     bias=bvs[:, 0:1], scale=1.0)
                nc.sync.dma_start(relt[:, rb * 512:(rb + 1) * 512], ot[:])

            # ---------------- NMS masks ----------------
            for ci in range(CLS_PER_CORE):
                pr = pairp.tile([6, 10, N_OBJ], bf16, tag="pair")
                nc.sync.dma_start(pr[:], pair[ci])
                for bi in range(_NBLK):
                    j0 = bi * _BLK
                    L = N_OBJ - j0
                    isl = slice(j0, j0 + _BLK)
                    jsl = slice(j0, N_OBJ)
                    pA = pps.tile([_BLK, 512], f32, tag="pps")
                    pB = pps.tile([_BLK, 512], f32, tag="pps")
                    pC = pps.tile([_BLK, 512], f32, tag="pps")
                    pD = pps.tile([_BLK, 512], f32, tag="pps")
                    pE = pps.tile([_BLK, 512], f32, tag="pps")
                    for s, t in enumerate((pA, pB, pC, pD, pE)):
                        nc.tensor.matmul(t[:, :L], pr[:, s, isl],
                                         pr[:, 5 + s, jsl],
                                         start=True, stop=True)
                    rB = work.tile([_BLK, 512], f32, tag="rB")
                    rD = work.tile([_BLK, 512], f32, tag="rD")
                    sx = work.tile([_BLK, 512], f32, tag="sx")
                    sy = work.tile([_BLK, 512], f32, tag="sy")
                    w13 = work.tile([_BLK, 512], f32, tag="w13")
                    hr = work.tile([_BLK, 512], f32, tag="hr")
                    p13 = work.tile([_BLK, 512], f32, tag="p13")
                    msk = work.tile([_BLK, 512], u8, tag="msk")
                    # DVE (PSUM-capable): clamps and combines
                    nc.vector.tensor_scalar(rB[:, :L], pB[:, :L], 0.0, None,
                                            Alu.max)
                    nc.vector.tensor_scalar(rD[:, :L], pD[:, :L], 0.0, None,
                                            Alu.max)
                    nc.vector.scalar_tensor_tensor(
                        sx[:, :L], pA[:, :L], 0.0, rB[:, :L],
                        op0=Alu.max, op1=Alu.add)
                    nc.vector.scalar_tensor_tensor(
                        sy[:, :L], pC[:, :L], 0.0, rD[:, :L],
                        op0=Alu.max, op1=Alu.add)
                    # ACT: fused affine+relu
                    nc.scalar.activation(w13[:, :L], sx[:, :L], Act.Relu,
                                         bias=bis[:, bi, ci, 0:1], scale=-1.3)
                    nc.scalar.activation(hr[:, :L], sy[:, :L], Act.Relu,
                                         bias=bis[:, bi, ci, 1:2], scale=-1.0)
                    # GPSIMD: SBUF-only product
                    nc.gpsimd.tensor_tensor(p13[:, :L], w13[:, :L], hr[:, :L],
                                            op=Alu.mult)
                    # DVE: threshold compare against PSUM E
                    nc.vector.tensor_tensor(msk[:, :L], p13[:, :L], pE[:, :L],
                                            op=Alu.is_gt)
                    nc.sync.dma_start(masks[ci, isl, jsl], msk[:, :L])

    nc.compile()
    return nc


def _get_nc():
    if "nc" not in _cache:
        _cache["nc"] = _build_nc()
    return _cache["nc"]


def _rank2_bank(lhs_vec, rhs_vec):
    """[6, 2, 512] block for out[i,j] = lhs_vec[i] + rhs_vec[j] via K=6."""
    out = np.zeros((6, 2, N_OBJ), np.float32)
    l0, l1, l2 = _split3_bf16(lhs_vec)
    r0, r1, r2 = _split3_bf16(rhs_vec)
    out[0, 0], out[1, 0], out[2, 0] = l0, l1, l2   # lhs pieces
    out[3, 0] = out[4, 0] = out[5, 0] = 1.0
    out[0, 1] = out[1, 1] = out[2, 1] = 1.0
    out[3, 1], out[4, 1], out[5, 1] = r0, r1, r2   # rhs pieces
    return out


def _host_prep(vr, W_vr, b_vr, boxes_per_cls):
    import ml_dtypes

    f16 = np.float16
    vrT = np.ascontiguousarray(vr.T)  # [4096, 16384] f32
    vh = vrT.astype(f16)
    vl = (vrT - vh.astype(np.float32)).astype(f16)
    vst_full = np.concatenate([vh, vl], axis=0)  # [8192, 16384] f16

    WT = np.ascontiguousarray(W_vr.T)  # [4096, 51]
    Wh = WT.astype(f16)
    Wl = (WT - Wh.astype(np.float32)).astype(f16)

    def _rearr(w):
        return np.ascontiguousarray(
            w.reshape(D // _BLK, _BLK, C_REL).transpose(1, 0, 2))

    wh_host, wl_host = _rearr(Wh), _rearr(Wl)
    bvr_host = np.ascontiguousarray(b_vr.reshape(C_REL, 1).astype(np.float32))

    cls_ids = np.minimum(1 + np.arange(N_CORES * CLS_PER_CORE), C_OBJ - 1)
    cls_ids = cls_ids.reshape(N_CORES, CLS_PER_CORE)

    b = boxes_per_cls.astype(np.float32)  # [512, 151, 4]
    x1, y1, x2, y2 = b[..., 0], b[..., 1], b[..., 2], b[..., 3]
    wi = x2 - x1 + np.float32(1)
    hi = y2 - y1 + np.float32(1)
    a3 = NMS_THRESH * (wi * hi)

    in_maps = []
    for k in range(N_CORES):
        pair_host = np.zeros((CLS_PER_CORE, 6, 10, N_OBJ), np.float32)
        bias_host = np.zeros((_BLK, _NBLK, CLS_PER_CORE, 2), np.float32)
        for t in range(CLS_PER_CORE):
            c = int(cls_ids[k, t])
            banks = (
                _rank2_bank(x2[:, c], -x2[:, c]),    # A = x2_i - x2_j
                _rank2_bank(-x1[:, c], x1[:, c]),    # B = x1_j - x1_i
                _rank2_bank(y2[:, c], -y2[:, c]),    # C
                _rank2_bank(-y1[:, c], y1[:, c]),    # D
                _rank2_bank(a3[:, c], a3[:, c]),     # E
            )
            for s, bk in enumerate(banks):
                pair_host[t, :, s, :] = bk[:, 0, :]
                pair_host[t, :, 5 + s, :] = bk[:, 1, :]
            bias_host[:, :, t, 0] = (np.float32(1.3) * wi[:, c]).reshape(
                _NBLK, _BLK).T
            bias_host[:, :, t, 1] = hi[:, c].reshape(_NBLK, _BLK).T
        vst_k = np.ascontiguousarray(
            vst_full[:, k * ROWS_PER_CORE:(k + 1) * ROWS_PER_CORE])
        in_maps.append({
            "vst": vst_k, "wh": wh_host, "wl": wl_host, "bvr": bvr_host,
            "pair": pair_host.astype(ml_dtypes.bfloat16),
            "bias": bias_host,
        })
    return in_maps, cls_ids


def _resolve_and_argmax(obj_logits, mask_sym):
    """Bit-exact reference ordering + greedy resolve + masked argmax."""
    import jax

    cpu = jax.devices("cpu")[0]
    with jax.default_device(cpu):
        import jax.numpy as jnp

        probs_j = jax.nn.softmax(jnp.asarray(obj_logits), axis=1)
        probs = np.asarray(probs_j)  # [512, 151] f32
        orders = np.asarray(jnp.argsort(-probs_j, axis=0))  # stable

    C, N = C_OBJ, N_OBJ
    Ms = np.empty((C - 1, N, N), bool)
    for c in range(1, C):
        o = orders[:, c]
        Ms[c - 1] = mask_sym[c][np.ix_(o, o)]
    supp = np.zeros((C - 1, N), bool)
    keep_sorted = np.zeros((C - 1, N), bool)
    for i in range(N):
        ki = ~supp[:, i]
        keep_sorted[:, i] = ki
        supp |= Ms[:, i, :] & ki[:, None]
    keeps = np.zeros((N, C), np.float32)
    for c in range(1, C):
        keeps[orders[:, c], c] = keep_sorted[c - 1]

    obj_preds = np.argmax((keeps * probs)[:, 1:], axis=1).astype(np.int32) + 1
    return obj_preds


def kernel(obj_logits, vr, boxes_per_cls, W_vr, b_vr):
    from concourse.bass_utils import run_bass_kernel_spmd

    obj_logits = np.asarray(obj_logits, np.float32)
    vr = np.asarray(vr, np.float32)
    boxes_per_cls = np.asarray(boxes_per_cls, np.float32)
    W_vr = np.asarray(W_vr, np.float32)
    b_vr = np.asarray(b_vr, np.float32)

    nc = _get_nc()
    in_maps, cls_ids = _host_prep(vr, W_vr, b_vr, boxes_per_cls)
    res = run_bass_kernel_spmd(nc, in_maps, list(range(N_CORES)))
    outs = res.results

    rel_dists = np.concatenate(
        [np.ascontiguousarray(outs[k]["relt"].T) for k in range(N_CORES)],
        axis=0).astype(np.float32)

    mask_sym = np.zeros((C_OBJ, N_OBJ, N_OBJ), bool)
    for k in range(N_CORES):
        mk = outs[k]["masks"]  # [19, 512, 512] u8, upper band written
        for t in range(CLS_PER_CORE):
            gid = 1 + k * CLS_PER_CORE + t
            if gid <= C_OBJ - 1:
                u = mk[t] != 0
                mask_sym[gid] = u | u.T

    obj_preds = _resolve_and_argmax(obj_logits, mask_sym)
    return obj_logits.copy(), obj_preds, rel_dists
